# revision 24
# baseline (speedup 1.0000x reference)
"""MultiHeadAttention kernel for 8 TRN2 NeuronCores.

Problem: x[2, 4096, 512], w_qkv[1536, 512], w_out[512, 512], 8 heads, dh=64.
Sharding: batch*heads across cores — core c handles batch b=c//4 and heads
(2*(c%4), 2*(c%4)+1). Each core computes the out-projection partial for its
two heads; the four partials per batch are summed to form the output.

Per-core algorithm (flash-attention style, everything stays on-chip):
  - QKV projection from a host-pre-transposed X^T with per-head-pair
    gathered/transposed weight slices (bf16 operands, fp32 PSUM accumulate).
  - Per 512-token group, loop over 32 key chunks of 128:
    S^T[j, t] matmuls for both heads row-packed on the PE (contraction d=64
    at partitions 0-63 / 64-127), one fused Exp over both heads' PSUM banks
    on ScalarE, then O^T accumulation with V augmented by a ones column so
    the softmax row-sums fall out of the same matmul (row 64).
  - Row-sum reciprocals round-trip through a DRAM scratch to transpose them
    from [1, t] into per-partition [t, 1] layout.
  - Out-projection per 128-token tile, with per-head 1/sum applied as
    per-partition tensor_scalar multiplies.

I/O is bf16 (host casts) to minimize per-core transfer bytes; accumulation
and normalization are fp32 on device. Partials are summed on-device with a
ReduceScatter over each batch's 4-core group, so each core only outputs a
quarter slice.
"""

import os

import numpy as np

B, T, E = 2, 4096, 512
H, DH = 8, 64
TG = 512          # token-group width (one PSUM bank of fp32)
NTG = T // TG     # 8 token groups
NJ = T // 128     # 32 key chunks
NEC = E // 128    # 4 contraction chunks for the projections

USE_RS = os.environ.get("MHA_RS", "1") == "1"  # device ReduceScatter of partials

_CACHE: dict = {}


def _build(use_rs=USE_RS):
    import concourse.bacc as bacc
    import concourse.mybir as mybir
    import concourse.tile as tile

    f32 = mybir.dt.float32
    bf16 = mybir.dt.bfloat16
    Exp = mybir.ActivationFunctionType.Exp
    mult = mybir.AluOpType.mult
    add = mybir.AluOpType.add

    nc = bacc.Bacc(None, target_bir_lowering=False)

    xtq = nc.dram_tensor("xtq", [E, T // 4], bf16, kind="ExternalInput")
    xtb = nc.dram_tensor("xtb", [E, T // 4], bf16)
    xtg = nc.dram_tensor("xtg", [4 * E, T // 4], bf16)
    wq = nc.dram_tensor("wq", [E, 128], bf16, kind="ExternalInput")
    wk = nc.dram_tensor("wk", [E, 128], bf16, kind="ExternalInput")
    wv = nc.dram_tensor("wv", [E, 128], bf16, kind="ExternalInput")
    wo = nc.dram_tensor("wo", [128, E], bf16, kind="ExternalInput")
    if use_rs:
        # After ReduceScatter over the 4-core batch group, each core holds
        # the summed quarter slice [T/4, E].
        out = nc.dram_tensor("out", [T // 4, E], bf16, kind="ExternalOutput")
        rs_in = nc.dram_tensor("rs_in", [T, E], f32)
        rs_out = nc.dram_tensor("rs_out", [T // 4, E], f32)
    else:
        out = nc.dram_tensor("out", [T, E], bf16, kind="ExternalOutput")
    sdram = nc.dram_tensor("sdram", [2, T], f32)

    with tile.TileContext(nc) as tc:
        with tc.tile_pool(name="res", bufs=1) as rp:
            xts = [rp.tile([128, T], bf16, tag=f"xt{c}", name=f"xt{c}")
                   for c in range(NEC)]
            wqs = rp.tile([128, NEC * 128], bf16, tag="wq")
            wks = rp.tile([128, NEC * 128], bf16, tag="wk")
            wvs = rp.tile([128, NEC * 128], bf16, tag="wv")
            wos = rp.tile([128, E], bf16, tag="wo")
            nc.sync.dma_start(xtb[:, :], xtq[:, :])
            nc.gpsimd.collective_compute(
                "AllGather",
                mybir.AluOpType.bypass,
                replica_groups=[[0, 1, 2, 3], [4, 5, 6, 7]],
                ins=[xtb[:, :]],
                outs=[xtg[:, :]],
            )
            xtv = xtg[:, :].rearrange("(g e) i -> e g i", g=4)
            for c in range(NEC):
                nc.sync.dma_start(
                    xts[c][:, :].rearrange("p (g i) -> p g i", g=4),
                    xtv[c * 128:(c + 1) * 128, :, :])
            for wsb, wdr in ((wqs, wq), (wks, wk), (wvs, wv)):
                nc.sync.dma_start(
                    wsb[:, :].rearrange("p (c m) -> p c m", m=128),
                    wdr[:, :].rearrange("(c p) m -> p c m", p=128))
            nc.sync.dma_start(wos[:, :], wo[:, :])

            qt = rp.tile([128, T], bf16, tag="qt")    # Q_cat^T  [dcat, t]
            kt = rp.tile([128, T], bf16, tag="kt")    # K_cat^T  [dcat, t]
            vt = rp.tile([128, NJ * 130], bf16, tag="vt")  # 64 h0 | 1 | 64 h1 | 1
            u = rp.tile([128, T], bf16, tag="u")      # U^T rows 0:64 h0, 64:128 h1
            rt = rp.tile([128, 2 * NTG * 4], f32, tag="rt")  # 1/sums

            vt3 = vt[:, :].rearrange("p (c w) -> p c w", w=130)
            nc.vector.memset(vt3[:, :, 64:65], 1.0)
            nc.vector.memset(vt3[:, :, 129:130], 1.0)

            # ---- QKV projection ----
            with tc.tile_pool(name="pqk", bufs=2, space="PSUM") as pj:
                for t in range(NTG):
                    for wsb, dst in ((wqs, qt), (wks, kt)):
                        ps = pj.tile([128, TG], f32, tag="pqk")
                        for c in range(NEC):
                            nc.tensor.matmul(
                                ps[:, :],
                                lhsT=wsb[:, c * 128:(c + 1) * 128],
                                rhs=xts[c][:, t * TG:(t + 1) * TG],
                                start=(c == 0), stop=(c == NEC - 1),
                            )
                        nc.vector.tensor_copy(
                            dst[:, t * TG:(t + 1) * TG], ps[:, :])
                for j in range(NJ):
                    pv = pj.tile([128, 128], f32, tag="pv")
                    for c in range(NEC):
                        nc.tensor.matmul(
                            pv[:, :],
                            lhsT=xts[c][:, j * 128:(j + 1) * 128],
                            rhs=wvs[:, c * 128:(c + 1) * 128],
                            start=(c == 0), stop=(c == NEC - 1),
                        )
                    dst = vt[:, j * 130:(j + 1) * 130].rearrange(
                        "p (g w) -> p g w", w=65)[:, :, 0:64]
                    nc.vector.tensor_copy(
                        dst, pv[:, :].rearrange("p (g d) -> p g d", d=64))

            # ---- attention ----
            with (
                tc.tile_pool(name="ps", bufs=2, space="PSUM") as ps_pool,
                tc.tile_pool(name="po", bufs=2, space="PSUM") as po_pool,
                tc.tile_pool(name="pt", bufs=3) as pt_pool,
                tc.tile_pool(name="rr", bufs=2) as rr_pool,
            ):
                for t in range(NTG):
                    o0 = po_pool.tile([65, TG], f32, tag="o0")
                    o1 = po_pool.tile([65, TG], f32, tag="o1")
                    for j in range(NJ):
                        s = ps_pool.tile([128, 2 * TG], f32, tag="s")
                        nc.tensor.matmul(
                            s[:, 0:TG],
                            lhsT=kt[0:64, j * 128:(j + 1) * 128],
                            rhs=qt[0:64, t * TG:(t + 1) * TG],
                        )
                        nc.tensor.matmul(
                            s[:, TG:2 * TG],
                            lhsT=kt[64:128, j * 128:(j + 1) * 128],
                            rhs=qt[64:128, t * TG:(t + 1) * TG],
                        )
                        p = pt_pool.tile([128, 2 * TG], bf16, tag="p")
                        nc.scalar.activation(p[:, :], s[:, :], Exp,
                                             scale=DH ** -0.5)
                        nc.tensor.matmul(
                            o0[:, :],
                            lhsT=vt[:, j * 130:j * 130 + 65],
                            rhs=p[:, 0:TG],
                            start=(j == 0), stop=(j == NJ - 1),
                        )
                        nc.tensor.matmul(
                            o1[:, :],
                            lhsT=vt[:, j * 130 + 65:j * 130 + 130],
                            rhs=p[:, TG:2 * TG],
                            start=(j == 0), stop=(j == NJ - 1),
                        )
                    nc.vector.tensor_copy(
                        u[0:64, t * TG:(t + 1) * TG], o0[0:64, :])
                    nc.vector.tensor_copy(
                        u[64:128, t * TG:(t + 1) * TG], o1[0:64, :])
                    for hi, oh in ((0, o0), (1, o1)):
                        rr = rr_pool.tile([1, TG], f32, tag="rr")
                        nc.vector.reciprocal(rr[0:1, :], oh[64:65, :])
                        nc.sync.dma_start(
                            sdram[hi:hi + 1, t * TG:(t + 1) * TG], rr[0:1, :])

            # ---- out projection ----
            nc.sync.dma_start(
                rt[:, 0:32], sdram[0:1, :].rearrange("b (a p) -> (b p) a", p=128))
            nc.sync.dma_start(
                rt[:, 32:64], sdram[1:2, :].rearrange("b (a p) -> (b p) a", p=128))
            with (
                tc.tile_pool(name="pr", bufs=2, space="PSUM") as pr_pool,
                tc.tile_pool(name="ob", bufs=3) as ob_pool,
            ):
                for tt in range(T // 128):
                    p0 = pr_pool.tile([128, E], f32, tag="p0")
                    p1 = pr_pool.tile([128, E], f32, tag="p1")
                    nc.tensor.matmul(
                        p0[:, :], lhsT=u[0:64, tt * 128:(tt + 1) * 128],
                        rhs=wos[0:64, :])
                    nc.tensor.matmul(
                        p1[:, :], lhsT=u[64:128, tt * 128:(tt + 1) * 128],
                        rhs=wos[64:128, :])
                    ob = ob_pool.tile([128, E], f32, tag="ob")
                    nc.vector.tensor_scalar_mul(ob[:, :], p0[:, :], rt[:, tt:tt + 1])
                    nc.vector.scalar_tensor_tensor(
                        ob[:, :], p1[:, :], rt[:, 32 + tt:33 + tt], ob[:, :],
                        op0=mult, op1=add)
                    if use_rs:
                        nc.sync.dma_start(rs_in[tt * 128:(tt + 1) * 128, :],
                                          ob[:, :])
                    else:
                        obh = ob_pool.tile([128, E], bf16, tag="obh")
                        nc.vector.tensor_copy(obh[:, :], ob[:, :])
                        nc.sync.dma_start(out[tt * 128:(tt + 1) * 128, :],
                                          obh[:, :])

            if use_rs:
                # Sum the four partials of each batch on-device; every core
                # keeps the quarter slice (c%4) of its batch's output.
                nc.gpsimd.collective_compute(
                    "ReduceScatter",
                    add,
                    replica_groups=[[0, 1, 2, 3], [4, 5, 6, 7]],
                    ins=[rs_in[:, :]],
                    outs=[rs_out[:, :]],
                )
                with tc.tile_pool(name="fin", bufs=3) as fin_pool:
                    for tt in range(T // 4 // 128):
                        ft = fin_pool.tile([128, E], f32, tag="ft")
                        nc.sync.dma_start(ft[:, :],
                                          rs_out[tt * 128:(tt + 1) * 128, :])
                        fh = fin_pool.tile([128, E], bf16, tag="fh")
                        nc.vector.tensor_copy(fh[:, :], ft[:, :])
                        nc.sync.dma_start(out[tt * 128:(tt + 1) * 128, :],
                                          fh[:, :])

    nc.finalize()
    return nc


def _to_bf16(a):
    """Fast float32 -> bfloat16 (round to nearest even) via bit ops."""
    import ml_dtypes

    u = np.ascontiguousarray(a, dtype=np.float32).view(np.uint32)
    r = ((u + 0x7FFF + ((u >> 16) & 1)) >> 16).astype(np.uint16)
    return r.view(ml_dtypes.bfloat16)


def _prep_core(x, w_qkv, w_out, core):
    b = core // 4
    h0 = 2 * (core % 4)
    h1 = h0 + 1

    def rows(k, h):
        return [d * 24 + k * 8 + h for d in range(DH)]

    wq = _to_bf16(np.ascontiguousarray(w_qkv[rows(0, h0) + rows(0, h1), :].T))
    wk = _to_bf16(np.ascontiguousarray(w_qkv[rows(1, h0) + rows(1, h1), :].T))
    wv = _to_bf16(np.ascontiguousarray(w_qkv[rows(2, h0) + rows(2, h1), :].T))
    cols = list(range(h0 * DH, (h0 + 1) * DH)) + list(range(h1 * DH, (h1 + 1) * DH))
    wo = _to_bf16(np.ascontiguousarray(w_out[:, cols].T))
    g = core % 4
    q = T // 4
    xtq = _to_bf16(np.ascontiguousarray(x[b, g * q:(g + 1) * q, :].T))
    return {"xtq": xtq, "wq": wq, "wk": wk, "wv": wv, "wo": wo}


def _assemble(outs):
    y = np.empty((B, T, E), np.float32)
    if USE_RS:
        q = T // 4
        for c in range(8):
            b, sl = c // 4, c % 4
            y[b, sl * q:(sl + 1) * q] = outs[c]
    else:
        y[0] = outs[0] + outs[1] + outs[2] + outs[3]
        y[1] = outs[4] + outs[5] + outs[6] + outs[7]
    return y


def _prep_core(x, w_qkv, w_out, core):
    b = core // 4
    h0 = 2 * (core % 4)
    h1 = h0 + 1

    def rows(k, h):
        return [d * 24 + k * 8 + h for d in range(DH)]

    wq = _to_bf16(np.ascontiguousarray(w_qkv[rows(0, h0) + rows(0, h1), :].T))
    wk = _to_bf16(np.ascontiguousarray(w_qkv[rows(1, h0) + rows(1, h1), :].T))
    wv = _to_bf16(np.ascontiguousarray(w_qkv[rows(2, h0) + rows(2, h1), :].T))
    cols = list(range(h0 * DH, (h0 + 1) * DH)) + list(range(h1 * DH, (h1 + 1) * DH))
    wo = _to_bf16(np.ascontiguousarray(w_out[:, cols].T))
    g = core % 4
    q = T // 4
    xtq = _to_bf16(np.ascontiguousarray(x[b, g * q:(g + 1) * q, :].T))
    return {"xtq": xtq, "wq": wq, "wk": wk, "wv": wv, "wo": wo}


def _assemble(outs):
    y = np.empty((B, T, E), np.float32)
    if USE_RS:
        q = T // 4
        for c in range(8):
            b, sl = c // 4, c % 4
            y[b, sl * q:(sl + 1) * q] = outs[c]
    else:
        y[0] = outs[0] + outs[1] + outs[2] + outs[3]
        y[1] = outs[4] + outs[5] + outs[6] + outs[7]
    return y


def _get_runner():
    """Build the Bass program once and wrap it in a persistent jitted
    shard_map executable (run_bass_kernel_spmd re-traces on every call)."""
    if "runner" in _CACHE:
        return _CACHE["runner"]

    import jax
    import concourse.mybir as mybir
    from jax.sharding import Mesh, PartitionSpec
    from jax.experimental.shard_map import shard_map
    from concourse.bass2jax import _bass_exec_p, partition_id_tensor

    nc = _CACHE.setdefault("nc", _build())
    n_cores = 8
    partition_name = (nc.partition_id_tensor.name
                      if nc.partition_id_tensor else None)
    in_names, out_names, out_avals = [], [], []
    for alloc in nc.m.functions[0].allocations:
        if not isinstance(alloc, mybir.MemoryLocationSet):
            continue
        name = alloc.memorylocations[0].name
        if alloc.kind == "ExternalInput":
            if name != partition_name:
                in_names.append(name)
        elif alloc.kind == "ExternalOutput":
            out_names.append(name)
            shape = tuple(alloc.tensor_shape)
            out_avals.append(
                jax.core.ShapedArray(shape, mybir.dt.np(alloc.dtype)))
    all_in_names = (in_names + out_names
                    + ([partition_name] if partition_name else []))

    def _body(*args):
        operands = list(args)
        if partition_name is not None:
            operands.append(partition_id_tensor())
        return tuple(_bass_exec_p.bind(
            *operands, out_avals=tuple(out_avals),
            in_names=tuple(all_in_names), out_names=tuple(out_names),
            lowering_input_output_aliases=(), sim_require_finite=True,
            sim_require_nnan=True, nc=nc))

    devices = jax.devices()[:n_cores]
    mesh = Mesh(np.asarray(devices), ("core",))
    sharded = jax.jit(
        shard_map(_body, mesh=mesh,
                  in_specs=(PartitionSpec("core"),) * (len(in_names)
                                                       + len(out_names)),
                  out_specs=(PartitionSpec("core"),) * len(out_names),
                  check_rep=False),
        keep_unused=True)
    zeros = [jax.device_put(np.zeros((n_cores * a.shape[0], *a.shape[1:]),
                                     a.dtype)) for a in out_avals]

    def run(in_maps, fp=None):
        # Keep the uploaded inputs resident on device across calls with
        # identical inputs (fp = content fingerprint); execution still runs
        # on every call.
        dev = _CACHE.get("dev_in")
        if fp is None or _CACHE.get("dev_fp") != fp or dev is None:
            concat_in = [np.concatenate([np.asarray(in_maps[c][nm])
                                         for c in range(n_cores)], axis=0)
                         for nm in in_names]
            dev = [jax.device_put(a) for a in concat_in]
            _CACHE["dev_in"] = dev
            _CACHE["dev_fp"] = fp
        outs = sharded(*dev, *zeros)
        o = np.asarray(outs[0]).reshape(n_cores, *out_avals[0].shape)
        return [o[c] for c in range(n_cores)]

    _CACHE["runner"] = run
    return run


def kernel(x, w_qkv, w_out):
    from concourse.bass_utils import run_bass_kernel_spmd

    x = np.asarray(x, dtype=np.float32)
    w_qkv = np.asarray(w_qkv, dtype=np.float32)
    w_out = np.asarray(w_out, dtype=np.float32)

    if "nc" not in _CACHE:
        _CACHE["nc"] = _build()
    nc = _CACHE["nc"]

    in_maps = [_prep_core(x, w_qkv, w_out, c) for c in range(8)]
    res = run_bass_kernel_spmd(nc, in_maps, list(range(8)))
    outs = [np.asarray(res.results[c]["out"]).astype(np.float32)
            for c in range(8)]
    return _assemble(outs)
